# revision 1
# baseline (speedup 1.0000x reference)
"""Trainium2 Bass kernel for nn_BinarizedLinear (ES population binary matvec).

Computes, for each direction d: out[d, o] = (sum_i W[d,o,i] * x[d,i]) > bias[d,o]
with W in {0,1} (f32), x in {0,1} (bool), bias f32.

Strategy (memory-bound problem; 1 GiB of W traffic dominates):
  - Shard the 64 directions across 8 NeuronCores (8 per core, 128 MiB each).
  - Stream W in 4 MiB contiguous SWDGE DMAs that cast f32 -> bf16 in flight
    ([128 part, 4 o-tiles, 2048 i], 6-slot ring). HBM-read side is unchanged
    (the binding resource); SBUF-write side halves. Measured: the cast DMA
    sustains the same ~400 GB/s as a plain HWDGE copy. The last 4 o-tiles
    stream as 1 MiB pieces so the final DVE bite after the last DMA is small.
  - One fused DVE scalar_tensor_tensor per o-tile:
      scratch = (W_tile * 1.0) * x_bcast ; act_col = sum(scratch)
    This is the only compute pass over the bulk data (~450 GB/s source rate,
    at/above the DMA delivery rate), so the kernel tracks the HBM roofline.
    0/1 values are exact in bf16 and the reduction accumulates in fp32, so
    the integer-vs-bias compare is bit-exact vs the f32 reference.
  - x rows (uint8) are partition-broadcast + cast to bf16 with stride-0-AP
    SWDGE DMAs, double-buffered per direction.
  - The 8x16 fp32 accumulator columns land in a [128,128] tile; PE transposes
    re-layout it so the bias load, is_gt compare and bool store are fully
    contiguous DMAs. Finalization is two-phase: directions 0-6 are
    transposed/compared/stored while direction 7 still streams; only the
    16-column tail runs after the last STT.
"""

from contextlib import ExitStack

import numpy as np

import concourse.bass as bass
from concourse import mybir
from concourse.bass_utils import run_bass_kernel_spmd

N_CORES = 8
D_TOT, OUT, IN = 64, 2048, 2048
D = D_TOT // N_CORES  # 8 directions per core
P = 128
NT = OUT // P         # 16 o-tiles per direction
CH = 4                # o-tiles per big W DMA (4 MiB transfers)
NTILE = D * NT        # 128 o-tiles = STTs
BUFS = 6              # W ring-buffer depth (bf16 slots, 16 KiB/partition)
NSCR = 4              # rotating STT scratch outputs
J1 = 7 * NT           # first 112 act columns = directions 0-6

# W stream segments (tile0, ntiles): the first and last 4 o-tiles go as
# 1 MiB pieces (fast DVE spin-up, small tail bite); the rest as 4 MiB chunks.
SEGS = (
    [(t, 1) for t in range(CH)]
    + [(t, CH) for t in range(CH, NTILE - CH, CH)]
    + [(t, 1) for t in range(NTILE - CH, NTILE)]
)
NDMA = len(SEGS)
# tiles consumed once segment k is fully processed
CUM = []
_c = 0
for _t0, _nt in SEGS:
    _c += _nt
    CUM.append(_c)


def build_program() -> bass.Bass:
    f32 = mybir.dt.float32
    bf16 = mybir.dt.bfloat16
    u8 = mybir.dt.uint8
    Alu = mybir.AluOpType

    nc = bass.Bass()
    w = nc.declare_dram_parameter("w", [D, OUT, IN], f32, isOutput=False)
    x = nc.declare_dram_parameter("x", [D, IN], u8, isOutput=False)
    b = nc.declare_dram_parameter("b", [D, OUT], f32, isOutput=False)
    o = nc.declare_dram_parameter("o", [D, OUT], u8, isOutput=True)

    # o = c*128 + p: partition p of o-tile c holds output row c*128+p.
    w_r = w[:].rearrange("d (c p) i -> d p c i", p=P)
    # [128, 128] views of bias/out matching the post-transpose layout:
    # partition j = d*16 + c, free f = p  ->  flat offset j*128 + f.
    bias_r = b[:].rearrange("d (h f) -> (d h) f", f=P)
    out_r = o[:].rearrange("d (h f) -> (d h) f", f=P)

    psum_t = nc.alloc_psum_tensor("psum_t", [P, P], f32)
    psum2 = nc.alloc_psum_tensor("psum2", [P - J1, P], f32)

    # Segment k -> (slot, ntiles, source AP slice). Each segment's tiles live
    # in columns 0..ntiles of slot k % BUFS; the head pieces (k < CH) go to
    # column k of the f32 staging slot (slot index -1), DMA'd via HWDGE on SP
    # which starts ~3 us before the first SWDGE emission.
    def wtile_src(k):
        t0, ntl = SEGS[k]
        d, c = divmod(t0, NT)
        if k < CH:
            return -1, ntl, w_r[d, :, c:c + ntl, :]
        return k % BUFS, ntl, w_r[d, :, c:c + ntl, :]

    with ExitStack() as ctx:
        wslots = [
            ctx.enter_context(nc.sbuf_tensor(f"w{s}", [P, CH, IN], bf16))
            for s in range(BUFS)
        ]
        wstage = ctx.enter_context(nc.sbuf_tensor("wstage", [P, CH, IN], f32))
        xbs = [
            ctx.enter_context(nc.sbuf_tensor(f"xb{s}", [P, IN], bf16))
            for s in range(2)
        ]
        scrs = [
            ctx.enter_context(nc.sbuf_tensor(f"scr{s}", [P, IN], bf16))
            for s in range(NSCR)
        ]
        dump2 = ctx.enter_context(nc.sbuf_tensor("dump2", [P, NTILE], bf16))
        bias_sb = ctx.enter_context(nc.sbuf_tensor("bias_sb", [P, P], f32))
        bias2_sb = ctx.enter_context(nc.sbuf_tensor("bias2_sb", [P - J1, P], f32))
        act_all = ctx.enter_context(nc.sbuf_tensor("act_all", [P, P], f32))
        ident = ctx.enter_context(nc.sbuf_tensor("ident", [P, P], f32))
        out_sb = ctx.enter_context(nc.sbuf_tensor("out_sb", [P, P], u8))
        out2_sb = ctx.enter_context(nc.sbuf_tensor("out2_sb", [P - J1, P], u8))

        block = ctx.enter_context(nc.Block())
        # One semaphore per DMA: each goes 0 -> 16 exactly once.
        wsem = [ctx.enter_context(nc.semaphore(f"wsem{k}")) for k in range(NDMA)]
        xsem = [ctx.enter_context(nc.semaphore(f"xsem{d}")) for d in range(D)]
        bias_sem = ctx.enter_context(nc.semaphore("bias_sem"))
        bias2_sem = ctx.enter_context(nc.semaphore("bias2_sem"))
        prod_sem = ctx.enter_context(nc.semaphore("prod_sem"))
        scr_sem = ctx.enter_context(nc.semaphore("scr_sem"))
        ident_sem = ctx.enter_context(nc.semaphore("ident_sem"))
        pe_sem = ctx.enter_context(nc.semaphore("pe_sem"))
        cmp_sem = ctx.enter_context(nc.semaphore("cmp_sem"))
        out_sem1 = ctx.enter_context(nc.semaphore("out_sem1"))
        out_sem2 = ctx.enter_context(nc.semaphore("out_sem2"))

        # Number of STTs completed once segment k has been fully consumed.
        def stts_done(k):
            return CUM[k]

        @block.gpsimd
        def _(gp):
            def issue_x(d):
                if d >= 2:
                    # slot d%2 was last used by direction d-2
                    gp.wait_ge(scr_sem, NT * (d - 1))
                xd = x[d:d + 1, :]
                bc = bass.AP(
                    tensor=xd.tensor,
                    offset=xd.offset,
                    ap=[[0, P], list(xd.ap)[-1]],
                )
                gp.dma_start(out=xbs[d % 2][:], in_=bc).then_inc(xsem[d], 16)

            # W streamer (SWDGE cast f32->bf16): ring of BUFS slots throttled
            # by DVE consumption; xb broadcasts interleaved ahead of need.
            issue_x(0)
            issue_x(1)
            for k in range(CH, NDMA):
                if k == 4:
                    # Identity for the PE transposes — emitted after the first
                    # W pieces are in flight so it doesn't delay stream start
                    # (PE only needs it much later, at scr_sem >= J1).
                    gp.memset(ident[:], 0.0).then_inc(ident_sem, 1)
                    gp.wait_ge(ident_sem, 1)
                    gp.affine_select(
                        out=ident[:],
                        in_=ident[:],
                        compare_op=Alu.not_equal,
                        fill=1.0,
                        base=0,
                        pattern=[[-1, P]],
                        channel_multiplier=1,
                    ).then_inc(ident_sem, 1)
                t0, _ntl = SEGS[k]
                dd, cc = divmod(t0, NT)
                if cc == CH and dd >= 2:
                    issue_x(dd)
                if k >= BUFS:
                    # slot reuse: segment k-BUFS must be fully consumed
                    gp.wait_ge(scr_sem, stts_done(k - BUFS))
                slot, ntl, src = wtile_src(k)
                gp.dma_start(
                    out=wslots[slot][:, 0:ntl, :], in_=src
                ).then_inc(wsem[k], 16)

        @block.vector
        def _(dve):
            # Pure bf16 multiply (2x_1P mode): prod = W_tile * x_bcast.
            n = 0  # global tile index
            for k in range(NDMA):
                slot, ntl, _src = wtile_src(k)
                if n % NT == 0:
                    dve.wait_ge(xsem[n // NT], 16)
                dve.wait_ge(wsem[k], 16)
                for c in range(ntl):
                    if n >= NSCR:
                        # scratch n%NSCR was consumed by ACT at tile n-NSCR
                        dve.wait_ge(scr_sem, n - (NSCR - 1))
                    src_tile = (
                        wstage[:, n, :] if slot == -1
                        else wslots[slot][:, c, :]
                    )
                    dve.tensor_tensor(
                        out=scrs[n % NSCR][:],
                        in0=src_tile,
                        in1=xbs[(n // NT) % 2][:],
                        op=Alu.mult,
                    ).then_inc(prod_sem, 1)
                    n += 1
                if n == J1:
                    # Directions 0-6 done: compare their transposed act
                    # against bias while direction 7 still streams.
                    dve.wait_ge(pe_sem, 1)
                    dve.wait_ge(bias_sem, 16)
                    dve.tensor_tensor(
                        out=out_sb[:J1, :],
                        in0=psum_t[:J1, :],
                        in1=bias_sb[:J1, :],
                        op=Alu.is_gt,
                    ).then_inc(cmp_sem, 1)
            # Tail: direction 7 only (16 columns).
            dve.wait_ge(pe_sem, 2)
            dve.wait_ge(bias2_sem, 16)
            dve.tensor_tensor(
                out=out2_sb[:], in0=psum2[:], in1=bias2_sb[:], op=Alu.is_gt
            ).then_inc(cmp_sem, 1)

        @block.scalar
        def _(act):
            act.dma_start(out=bias_sb[:J1, :], in_=bias_r[:J1, :]).then_inc(
                bias_sem, 16
            )
            act.dma_start(out=bias2_sb[:], in_=bias_r[J1:, :]).then_inc(
                bias2_sem, 16
            )
            # Accumulating reduce of each product tile: act_col = sum(prod).
            for n in range(NTILE):
                act.wait_ge(prod_sem, n + 1)
                act.activation(
                    out=dump2[:, n:n + 1].broadcast_to([P, IN]),
                    in_=scrs[n % NSCR][:],
                    func=mybir.ActivationFunctionType.Copy,
                    accum_out=act_all[:, n:n + 1],
                ).then_inc(scr_sem, 1)

        @block.tensor
        def _(pe):
            pe.wait_ge(ident_sem, 2)
            pe.wait_ge(scr_sem, J1)
            pe.transpose(psum_t[:J1, :], act_all[:, :J1], ident[:]).then_inc(
                pe_sem, 1
            )
            pe.wait_ge(scr_sem, NTILE)
            pe.transpose(psum2[:], act_all[:, J1:], ident[:]).then_inc(pe_sem, 1)

        @block.sync
        def _(sp):
            # Head pieces via HWDGE — the earliest data on the wire.
            for pc in range(CH):
                sp.dma_start(
                    out=wstage[:, pc:pc + 1, :], in_=w_r[0, :, pc:pc + 1, :]
                ).then_inc(wsem[pc], 16)
            sp.wait_ge(cmp_sem, 1)
            sp.dma_start(out=out_r[:J1, :], in_=out_sb[:J1, :]).then_inc(
                out_sem1, 16
            )
            sp.wait_ge(cmp_sem, 2)
            sp.dma_start(out=out_r[J1:, :], in_=out2_sb[:]).then_inc(out_sem2, 16)
            sp.wait_ge(out_sem1, 16)
            sp.wait_ge(out_sem2, 16)

    return nc


_prog = None


def _get_prog() -> bass.Bass:
    global _prog
    if _prog is None:
        _prog = build_program()
    return _prog


def make_in_maps(weight_noise, x, bias_noise):
    w = np.ascontiguousarray(weight_noise, dtype=np.float32)
    xf = np.ascontiguousarray(x).astype(np.uint8)
    bf = np.ascontiguousarray(bias_noise, dtype=np.float32)
    in_maps = []
    for c in range(N_CORES):
        sl = slice(c * D, (c + 1) * D)
        in_maps.append({"w": w[sl], "x": xf[sl], "b": bf[sl]})
    return in_maps


def kernel(**inputs) -> np.ndarray:
    nc = _get_prog()
    in_maps = make_in_maps(
        inputs["weight_noise"], inputs["x"], inputs["bias_noise"]
    )
    res = run_bass_kernel_spmd(nc, in_maps, list(range(N_CORES)))
    outs = [res.results[c]["o"] for c in range(N_CORES)]
    return np.concatenate(outs, axis=0).astype(bool)



# revision 8
# speedup vs baseline: 5.1865x; 5.1865x over previous
"""Trainium2 Bass kernel for nn_BinarizedLinear (ES population binary matvec).

Computes, for each direction d: out[d, o] = (sum_i W[d,o,i] * x[d,i]) > bias[d,o]
with W in {0,1} (f32), x in {0,1} (bool), bias f32.

Strategy (v3 — bit-packed popcount; the original nn.Module stores these
weights bit-packed in int64 words, so the host packs them the same way):
  - Host packs W bits 8-per-byte: HBM traffic drops 32x vs f32
    (134 MiB -> 4.2 MiB per core; the f32 baseline was DMA-bound ~350 us).
  - Layout per direction: u16[j, o] = byte(B=2j) | byte(B=2j+1)<<8 where
    byte B holds input bits i = 8B..8B+7 of output row o. One SBUF
    partition j owns input-byte pair (2j, 2j+1) for all 2048 o.
  - DVE (the Pool engine has no integer ALU, so the whole SWAR chain runs
    on DVE; tensor_scalar imm ops run in 4x_2p mode, tensor_tensor in 2x_1p):
      z = Wp & x_dup      (TT; x stored as duplicated u16 pairs so the
                           broadcast AP keeps a stride-1 last dim)
      s = (z >> 1) & 0x5555                  (TS imm)
      u = z - s           (TT) -> SWAR pair-counts {0,1,2} in 2-bit fields
      plane_f = (u << {3,1} | >> {1,3}) & 0x1818   (TS imm, 4 planes)
    Plane byte values {0x00,0x08,0x10} read as fp8e4m3 are {0, 2^-6, 2^-5}:
    exactly linear in the count (normal fp8 range, no subnormals).
  - PE reduces each plane over j with fp8 DoubleRow matmuls (2 k-tiles of
    128 partitions = all 256 bytes per contraction): stationary is a
    one-hot column of 64.0 at slot d*4+c, so psum[32, 512] row d*4+c
    accumulates act[d, c*512:(c+1)*512] = sum of counts exactly
    (64 * 2^-6 * count = count; integer f32 accumulation, bit-exact).
  - Tail: one DVE is_gt psum vs bias (host pre-arranged [32, 512]) -> u8 out.
"""

from contextlib import ExitStack

import numpy as np

import concourse.bass as bass
from concourse import mybir
from concourse.bass_utils import run_bass_kernel_spmd

N_CORES = 8
D_TOT, OUT, IN = 64, 2048, 2048
D = D_TOT // N_CORES  # 8 directions per core
P = 128
NCH = 4               # o-chunks of 512 (psum free width)
CHW = OUT // NCH      # 512
NPL = 4               # pair planes per direction
OPD = 3 + NPL         # DVE ops per direction (z, s, u, 4 extracts)
# plane f covers bit-pair (2f, 2f+1) of each byte; the shift moves the
# 2-bit field to bits 3-4 (fp8e4 values {0, 2^-6, 2^-5}, linear in count).
PLANE_SHIFTS = [(mybir.AluOpType.logical_shift_left, 3),
                (mybir.AluOpType.logical_shift_left, 1),
                (mybir.AluOpType.logical_shift_right, 1),
                (mybir.AluOpType.logical_shift_right, 3)]
PLANE_MASK = 0x1818
STAT_HOT = 32         # one-hot column position in statw


def _pair_view(ap):
    """[P, N] -> [P, N/2, 2] AP (free dims split so the last dim is a
    stride-1 pair, matching the broadcast x operand)."""
    return ap.rearrange("p (a b) -> p a b", b=2)


def build_program() -> bass.Bass:
    f32 = mybir.dt.float32
    f8 = mybir.dt.float8e4
    u16 = mybir.dt.uint16
    u8 = mybir.dt.uint8
    Alu = mybir.AluOpType
    DR = mybir.MatmulPerfMode.DoubleRow

    nc = bass.Bass()
    wp = nc.declare_dram_parameter("wp", [D, P, OUT], u16, isOutput=False)
    xaux = nc.declare_dram_parameter("xaux", [P, 2 * D], u16, isOutput=False)
    b = nc.declare_dram_parameter("b", [4 * D, CHW], f32, isOutput=False)
    o = nc.declare_dram_parameter("o", [4 * D, CHW], u8, isOutput=True)

    psum = nc.alloc_psum_tensor("psum", [4 * D, CHW], f32)

    with ExitStack() as ctx:
        wbufs = [
            ctx.enter_context(nc.sbuf_tensor(f"w{s}", [P, OUT], u16))
            for s in range(3)
        ]
        zb = [
            ctx.enter_context(nc.sbuf_tensor(f"z{s}", [P, OUT], u16))
            for s in range(2)
        ]
        sb = [
            ctx.enter_context(nc.sbuf_tensor(f"s{s}", [P, OUT], u16))
            for s in range(2)
        ]
        ub = [
            ctx.enter_context(nc.sbuf_tensor(f"u{s}", [P, OUT], u16))
            for s in range(2)
        ]
        planes = [
            ctx.enter_context(nc.sbuf_tensor(f"pl{s}", [P, NPL, OUT], u16))
            for s in range(2)
        ]
        xa_sb = ctx.enter_context(nc.sbuf_tensor("xa_sb", [P, 2 * D], u16))
        statw = ctx.enter_context(nc.sbuf_tensor("statw", [P, 2, 64], f8))
        bias_sb = ctx.enter_context(nc.sbuf_tensor("bias_sb", [4 * D, CHW], f32))
        outc = ctx.enter_context(nc.sbuf_tensor("outc", [4 * D, CHW], u8))

        block = ctx.enter_context(nc.Block())
        wsem = [
            ctx.enter_context(nc.semaphore(f"wsem{d}")) for d in range(D)
        ]
        xsem = ctx.enter_context(nc.semaphore("xsem"))
        bsem = ctx.enter_context(nc.semaphore("bsem"))
        dv_sem = ctx.enter_context(nc.semaphore("dv_sem"))
        pe_sem = ctx.enter_context(nc.semaphore("pe_sem"))
        st_sem = ctx.enter_context(nc.semaphore("st_sem"))
        cmp_sem = ctx.enter_context(nc.semaphore("cmp_sem"))
        out_sem = ctx.enter_context(nc.semaphore("out_sem"))

        @block.sync
        def _(sp):
            for d in range(D):
                if d >= 3:
                    # wbuf slot d%3 free once z of direction d-3 was read
                    sp.wait_ge(dv_sem, OPD * (d - 3) + 1)
                sp.dma_start(out=wbufs[d % 3][:], in_=wp[d]).then_inc(
                    wsem[d], 16
                )

        @block.scalar
        def _(act):
            act.dma_start(out=xa_sb[:], in_=xaux[:]).then_inc(xsem, 16)
            act.dma_start(out=bias_sb[:], in_=b[:]).then_inc(bsem, 16)
            act.wait_ge(cmp_sem, 1)
            act.dma_start(out=o[:], in_=outc[:]).then_inc(out_sem, 16)
            act.wait_ge(out_sem, 16)

        @block.gpsimd
        def _(gp):
            gp.memset(statw[:], 0.0).then_inc(st_sem, 1)
            gp.wait_ge(st_sem, 1)
            gp.memset(statw[:, :, STAT_HOT:STAT_HOT + 1], 64.0).then_inc(
                st_sem, 1
            )

        @block.vector
        def _(dve):
            for d in range(D):
                s = d % 2
                if d == 0:
                    dve.wait_ge(xsem, 16)
                dve.wait_ge(wsem[d], 16)
                if d >= 1:
                    # z/s ring slots (d%2) free once u of direction d-1 ran
                    dve.wait_ge(dv_sem, OPD * (d - 1) + 3)
                xbc = _pair_view(xa_sb[:, 2 * d:2 * d + 2]).broadcast_to(
                    [P, OUT // 2, 2]
                )
                dve.tensor_tensor(
                    out=_pair_view(zb[s][:]),
                    in0=_pair_view(wbufs[d % 3][:]),
                    in1=xbc,
                    op=Alu.bitwise_and,
                ).then_inc(dv_sem, 1)
                dve.wait_ge(dv_sem, OPD * d + 1)
                dve.tensor_scalar(
                    out=sb[s][:], in0=zb[s][:],
                    scalar1=1, scalar2=0x5555,
                    op0=Alu.logical_shift_right, op1=Alu.bitwise_and,
                ).then_inc(dv_sem, 1)
                dve.wait_ge(dv_sem, OPD * d + 2)
                dve.tensor_tensor(
                    out=ub[s][:], in0=zb[s][:], in1=sb[s][:], op=Alu.subtract
                ).then_inc(dv_sem, 1)
                dve.wait_ge(dv_sem, OPD * d + 3)
                if d >= 2:
                    # planes slot reuse: PE consumed direction d-2's planes
                    dve.wait_ge(pe_sem, d - 1)
                for f in range(NPL):
                    op0, sh = PLANE_SHIFTS[f]
                    dve.tensor_scalar(
                        out=planes[s][:, f, :], in0=ub[s][:],
                        scalar1=sh, scalar2=PLANE_MASK,
                        op0=op0, op1=Alu.bitwise_and,
                    ).then_inc(dv_sem, 1)
            dve.wait_ge(pe_sem, D)
            dve.wait_ge(bsem, 16)
            dve.tensor_tensor(
                out=outc[:], in0=psum[:], in1=bias_sb[:], op=Alu.is_gt
            ).then_inc(cmp_sem, 1)

        @block.tensor
        def _(pe):
            pe.wait_ge(st_sem, 2)
            for d in range(D):
                s = d % 2
                # fp8 view of the plane buffer: byte index 2*o + bytepair
                pl8 = planes[s][:].bitcast(f8)  # [P, NPL, 2*OUT]
                for c in range(NCH):
                    hot = STAT_HOT - (4 * d + c)
                    lhsT = statw[:, :, hot:hot + 32]
                    for f in range(NPL):
                        if c == 0:
                            pe.wait_ge(dv_sem, OPD * d + 4 + f)
                        rhs = pl8[:, f, :].rearrange(
                            "p (o t) -> p t o", t=2
                        )[:, :, c * CHW:(c + 1) * CHW]
                        mm = pe.matmul(
                            out=psum[:],
                            lhsT=lhsT,
                            rhs=rhs,
                            start=(d == 0 and c == 0 and f == 0),
                            stop=(d == D - 1 and c == NCH - 1 and f == NPL - 1),
                            perf_mode=DR,
                        )
                        if c == NCH - 1 and f == NPL - 1:
                            mm.then_inc(pe_sem, 1)

    return nc


_prog = None


def _get_prog() -> bass.Bass:
    global _prog
    if _prog is None:
        _prog = build_program()
    return _prog


def _pack_core(w, x, bias):
    """Build one core's input map from its [D, OUT, IN] f32 / [D, IN] bool /
    [D, OUT] f32 shard."""
    pb = np.packbits(
        np.ascontiguousarray(w) != 0, axis=-1, bitorder="little"
    )  # [D, OUT, IN/8]
    # u16[j, o] = byte(2j, o) | byte(2j+1, o) << 8
    arr = pb.transpose(0, 2, 1).reshape(D, P, 2, OUT)  # [d, j, bpair, o]
    wp16 = np.ascontiguousarray(arr.transpose(0, 1, 3, 2)).view(np.uint16)
    wp16 = wp16.reshape(D, P, OUT)

    xb = np.packbits(np.ascontiguousarray(x), axis=-1, bitorder="little")
    xw = np.ascontiguousarray(xb.reshape(D, P, 2)).view(np.uint16).reshape(D, P)
    # duplicated pairs: cols (2d, 2d+1) both hold xw[d]
    xaux = np.repeat(xw.T, 2, axis=1).astype(np.uint16)  # [P, 2D]
    xaux = np.ascontiguousarray(xaux)

    br = np.ascontiguousarray(bias.astype(np.float32).reshape(D * NCH, CHW))
    return {"wp": wp16, "xaux": xaux, "b": br}


def make_in_maps(weight_noise, x, bias_noise):
    in_maps = []
    for c in range(N_CORES):
        sl = slice(c * D, (c + 1) * D)
        in_maps.append(_pack_core(weight_noise[sl], x[sl], bias_noise[sl]))
    return in_maps


def kernel(**inputs) -> np.ndarray:
    nc = _get_prog()
    in_maps = make_in_maps(
        inputs["weight_noise"], inputs["x"], inputs["bias_noise"]
    )
    res = run_bass_kernel_spmd(nc, in_maps, list(range(N_CORES)))
    outs = [res.results[c]["o"].reshape(D, OUT) for c in range(N_CORES)]
    return np.concatenate(outs, axis=0).astype(bool)
